# revision 17
# baseline (speedup 1.0000x reference)
"""DYSPN attention-conv kernel for Trainium2 (8 NeuronCores, batch-parallel).

Row-pair layout (partition p = image rows {2p,2p+1}); affinity host-repacked
column-major (chunk = upper/lower half of one 7x7-window column) and padded
to [128, cn, 2, 264]. DVE: z = att*aff -> fp8e4m3. ACT: |z| -> fp8.
PE: U/A/T fp8 band matmuls, paired taps via DoubleRow (2 contraction
rows/cycle, stacked per-tap band shifts). DVE: deferred epilogue with
approximate reciprocal. out = r*((T+att3)*cs - (U+att3)*co) + co,
r = 1/(A+att3+eps)."""
import sys

sys.path.insert(0, "/opt/trn_rl_repo")

import numpy as np

import concourse.bass as bass  # noqa: F401
import concourse.tile as tile
from concourse import bacc, mybir
from concourse.bass_utils import run_bass_kernel_spmd

FP32 = mybir.dt.float32
FP32R = mybir.dt.float32r
FP8 = mybir.dt.float8e4
DR = mybir.MatmulPerfMode.DoubleRow

N_CORES = 8
B_FULL = 16
B_CORE = B_FULL // N_CORES
H = W = 256
K = 7
PAD = 4
HWP = 2 * PAD + W
BANDW = 136
C0 = 4
EPS = 1e-6
EPI_AT = 3
EPI2_AT = 8

_INDEX = np.array([0, 0, 0, 0, 0, 0, 0,
                   0, 1, 1, 1, 1, 1, 0,
                   0, 1, 2, 2, 2, 1, 0,
                   0, 1, 2, 3, 2, 1, 0,
                   0, 1, 2, 2, 2, 1, 0,
                   0, 1, 1, 1, 1, 1, 0,
                   0, 0, 0, 0, 0, 0, 0], dtype=np.int64).reshape(7, 7)

CHUNK_DEFS = []
for j in range(K):
    CHUNK_DEFS.append((j, [0, 1, 2]))
    CHUNK_DEFS.append((j, [i for i in [3, 4, 5, 6] if not (i == 3 and j == 3)]))
CHUNK_TAPS = [[(i, int(_INDEX[i, j]), 3 - i) for i in rows]
              for j, rows in CHUNK_DEFS]
N3 = sum(1 for c in CHUNK_TAPS if len(c) == 3)
N4 = sum(1 for c in CHUNK_TAPS if len(c) == 4)


def _band_matrix() -> np.ndarray:
    band = np.zeros((128, BANDW), dtype=np.float32)
    for p in range(128):
        band[p, p + C0] = 1.0
    return band


def _build():
    nc = bacc.Bacc("TRN2", target_bir_lowering=False, debug=False,
                   num_devices=N_CORES)
    aff3 = nc.dram_tensor("aff3", [B_CORE, N3, 128, 3, 2, HWP], FP32,
                          kind="ExternalInput").ap()
    aff4 = nc.dram_tensor("aff4", [B_CORE, N4, 128, 4, 2, HWP], FP32,
                          kind="ExternalInput").ap()
    att = nc.dram_tensor("attention", [B_CORE, 128, 4, 2, HWP], FP32,
                         kind="ExternalInput").ap()
    cs = nc.dram_tensor("current_segmentation", [B_CORE, 128, 2, W], FP32,
                        kind="ExternalInput").ap()
    co = nc.dram_tensor("coarse_segmentation", [B_CORE, 128, 2, W], FP32,
                        kind="ExternalInput").ap()
    band = nc.dram_tensor("band", [128, BANDW], FP32, kind="ExternalInput").ap()
    out = nc.dram_tensor("out", [B_CORE, 128, 2, W], FP32,
                         kind="ExternalOutput").ap()

    with tile.TileContext(nc) as tc:
        with tc.tile_pool(name="const", bufs=1) as cpool, \
             tc.tile_pool(name="a3", bufs=8) as ap3, \
             tc.tile_pool(name="a4", bufs=7) as ap4, \
             tc.tile_pool(name="f3", bufs=8) as fp3, \
             tc.tile_pool(name="f4", bufs=7) as fp4, \
             tc.tile_pool(name="azp", bufs=3) as azpool, \
             tc.tile_pool(name="inp", bufs=2) as ipool, \
             tc.tile_pool(name="ep", bufs=2) as epool, \
             tc.tile_pool(name="ps", bufs=2, space="PSUM") as pspool:

            bandf = cpool.tile([128, BANDW], FP32)
            nc.scalar.dma_start(out=bandf[:], in_=band[:, :])
            bandr = cpool.tile([128, BANDW], FP32R)
            nc.vector.tensor_copy(bandr[:], bandf[:])
            identr = bandr[:, C0:C0 + 128]
            bandf8 = cpool.tile([128, BANDW], FP8)
            nc.vector.tensor_copy(bandf8[:], bandf[:])

            def ident8(s):
                return bandf8[:, C0 + s:C0 + s + 128]

            wpairs = {}
            for sp in [(0, 0), (1, 0), (2, 1), (-1, -2), (0, -1)]:
                wt = cpool.tile([128, 2, 128], FP8,
                                tag=f"w{sp[0]}_{sp[1]}")
                nc.vector.tensor_copy(wt[:, 0],
                                      bandf[:, C0 + sp[0]:C0 + sp[0] + 128])
                nc.vector.tensor_copy(wt[:, 1],
                                      bandf[:, C0 + sp[1]:C0 + sp[1] + 128])
                wpairs[sp] = wt

            st = {}

            def epilogue1(img, pool=False):
                # pool=True runs the elementwise chain on the otherwise-idle
                # Pool engine (via an ACT PSUM->SBUF copy of U, Pool has no
                # PSUM port) to take load off the bottleneck DVE stream;
                # only safe when epilogue2 is deferred far enough that the
                # slow Pool ops cannot head-of-line-block the DVE queue.
                s = st[img]
                e = epool.tile([128, 2, W], FP32, tag="e")
                s["e"] = e
                nc.scalar.activation(e[:], s["A"][:],
                                     mybir.ActivationFunctionType.Copy,
                                     bias=EPS)
                nc.vector.reciprocal_approx_fast(e[:], e[:])
                m2 = epool.tile([128, 2, W], FP32, tag="m2")
                s["m2"] = m2
                if pool:
                    u = epool.tile([128, 2, W], FP32, tag="u")
                    nc.scalar.activation(u[:], s["U"][:],
                                         mybir.ActivationFunctionType.Copy)
                    nc.gpsimd.tensor_mul(m2[:], u[:], s["cot"][:])
                    nc.gpsimd.tensor_mul(m2[:], m2[:], e[:])
                    nc.gpsimd.tensor_sub(m2[:], m2[:], s["cot"][:])
                    nc.gpsimd.tensor_mul(e[:], e[:], s["cst"][:])
                else:
                    nc.vector.tensor_mul(m2[:], s["U"][:], s["cot"][:])
                    nc.vector.tensor_mul(m2[:], m2[:], e[:])
                    nc.vector.tensor_sub(m2[:], m2[:], s["cot"][:])
                    # csr = cs/denom now (only needs e + cs) so epilogue2
                    # is two short ops per half, overlapped with the store
                    nc.vector.tensor_mul(e[:], e[:], s["cst"][:])

            def epilogue2(img):
                s = st[img]
                e, m2 = s["e"], s["m2"]
                m1 = epool.tile([128, 2, W], FP32, tag="m1")
                for h in (0, 1):
                    nc.vector.tensor_mul(m1[:, h], s["T"][:, h], e[:, h])
                    nc.vector.tensor_sub(m1[:, h], m1[:, h], m2[:, h])
                    nc.scalar.dma_start(out=out[img, :, h], in_=m1[:, h])

            attfs = []
            for img in range(B_CORE):
                attf = ipool.tile([128, 4, 2, HWP], FP32, tag="attf")
                nc.gpsimd.dma_start(out=attf[:], in_=att[img])
                cst = ipool.tile([128, 2, W], FP32, tag="cst")
                nc.gpsimd.dma_start(out=cst[:], in_=cs[img])
                cot = ipool.tile([128, 2, W], FP32, tag="cot")
                nc.gpsimd.dma_start(out=cot[:], in_=co[img])
                att3r = ipool.tile([128, 2, W], FP32R, tag="att3r")
                nc.scalar.activation(att3r[:], attf[:, 3, :, PAD:PAD + W],
                                     mybir.ActivationFunctionType.Copy)
                attfs.append(attf)
                psU = pspool.tile([128, 2, W], FP32, tag="U")
                psA = pspool.tile([128, 2, W], FP32, tag="A")
                psT = pspool.tile([128, 2, W], FP32, tag="T")
                st[img] = {"U": psU, "A": psA, "T": psT, "cst": cst,
                           "cot": cot}
                nc.tensor.matmul(out=psU[:], lhsT=identr, rhs=att3r[:],
                                 start=True, stop=False)
                nc.tensor.matmul(out=psA[:], lhsT=identr, rhs=att3r[:],
                                 start=True, stop=False)
                nc.tensor.matmul(out=psT[:], lhsT=identr, rhs=att3r[:],
                                 start=True, stop=False)

            for img in range(B_CORE):
                attf = attfs[img]
                psU = st[img]["U"]
                psA = st[img]["A"]
                psT = st[img]["T"]

                afts, zf8s = [], []
                i3 = i4 = 0
                for ci, taps in enumerate(CHUNK_TAPS):
                    if len(taps) == 3:
                        aft = ap3.tile([128, 3, 2, HWP], FP32, tag="a")
                        zf8 = fp3.tile([128, 3, 2, HWP], FP8, tag="f")
                        nc.sync.dma_start(out=aft[:], in_=aff3[img, i3])
                        i3 += 1
                    else:
                        aft = ap4.tile([128, 4, 2, HWP], FP32, tag="a")
                        zf8 = fp4.tile([128, 4, 2, HWP], FP8, tag="f")
                        nc.sync.dma_start(out=aft[:], in_=aff4[img, i4])
                        i4 += 1
                    afts.append(aft)
                    zf8s.append(zf8)

                for ci, taps in enumerate(CHUNK_TAPS):
                    j, _ = CHUNK_DEFS[ci]
                    dx = 3 - j
                    cn = len(taps)
                    aft, zf8 = afts[ci], zf8s[ci]
                    final = ci == len(CHUNK_TAPS) - 1
                    a = 0
                    while a < cn:
                        b = a + 1
                        while b < cn and taps[b][1] == taps[a][1]:
                            b += 1
                        r = taps[a][1]
                        nc.vector.tensor_tensor(
                            out=zf8[:, a:b],
                            in0=aft[:, a:b],
                            in1=attf[:, r].unsqueeze(1).broadcast_to(
                                [128, b - a, 2, HWP]),
                            op=mybir.AluOpType.mult)
                        a = b
                    azt = azpool.tile([128, 4, 2, W], FP8, tag="az")
                    nc.scalar.activation(azt[:, 0:cn],
                                         zf8[:, :, :, PAD:PAD + W],
                                         mybir.ActivationFunctionType.Abs)

                    def sum_pairs(ps, src, off, stop, cn=cn):
                        mms = []
                        for a0 in range(0, cn - 1, 2):
                            for h in (0, 1):
                                mms.append((ps[:, h], wpairs[(0, 0)][:],
                                            src[:, a0:a0 + 2, h,
                                                off:off + W], DR))
                        if cn % 2:
                            mms.append((ps[:, :, :], ident8(0),
                                        src[:, cn - 1, :, off:off + W],
                                        None))
                        for n, (o, w, rhs, pm) in enumerate(mms):
                            nc.tensor.matmul(out=o, lhsT=w, rhs=rhs,
                                             perf_mode=pm, start=False,
                                             stop=(stop and n == len(mms) - 1),
                                             skip_group_check=True)

                    def mm_t(stop, taps=taps, dx=dx, zf8=zf8):
                        mms = []
                        odd = [n for n, (i, r, dy) in enumerate(taps)
                               if dy % 2]
                        ev = [n for n, (i, r, dy) in enumerate(taps)
                              if dy % 2 == 0]
                        xo = PAD + dx
                        if dx % 2 == 0:
                            for h in (0, 1):
                                sab = tuple((taps[n][2] - 1) // 2 if h == 0
                                            else (taps[n][2] + 1) // 2
                                            for n in odd)
                                mms.append((psT[:, h], wpairs[sab][:],
                                            zf8[:, odd[0]:odd[1] + 1:2,
                                                1 - h, xo:xo + W], DR))
                            if len(ev) == 2:
                                sab = tuple(taps[n][2] // 2 for n in ev)
                                for h in (0, 1):
                                    mms.append((psT[:, h], wpairs[sab][:],
                                                zf8[:, ev[0]:ev[1] + 1:2,
                                                    h, xo:xo + W], DR))
                            else:
                                s = taps[ev[0]][2] // 2
                                mms.append((psT[:, :, :], ident8(s),
                                            zf8[:, ev[0], :, xo:xo + W],
                                            None))
                        else:
                            for n, (i, r, dy) in enumerate(taps):
                                if dy % 2 == 0:
                                    mms.append((psT[:, :, :],
                                                ident8(dy // 2),
                                                zf8[:, n, :, xo:xo + W],
                                                None))
                                else:
                                    for h in (0, 1):
                                        s = ((dy - 1) // 2 if h == 0
                                             else (dy + 1) // 2)
                                        mms.append((psT[:, h], ident8(s),
                                                    zf8[:, n, 1 - h,
                                                        xo:xo + W], None))
                        for n, (o, w, rhs, pm) in enumerate(mms):
                            nc.tensor.matmul(out=o, lhsT=w, rhs=rhs,
                                             perf_mode=pm, start=False,
                                             stop=(stop and n == len(mms) - 1),
                                             skip_group_check=True)

                    if final and img == B_CORE - 1:
                        sum_pairs(psU, zf8, PAD, True)
                        sum_pairs(psA, azt, 0, True)
                        epilogue1(img)
                        mm_t(True)
                        epilogue2(img)
                    else:
                        sum_pairs(psU, zf8, PAD, final)
                        mm_t(final)
                        sum_pairs(psA, azt, 0, final)
                    if ci == EPI_AT and img > 0:
                        epilogue1(img - 1, pool=True)
                    if ci == EPI2_AT and img > 0:
                        epilogue2(img - 1)

    nc.compile()
    return nc


_NC_CACHE = None


def _get_nc():
    global _NC_CACHE
    if _NC_CACHE is None:
        _NC_CACHE = _build()
    return _NC_CACHE


def run(inputs: dict, trace: bool = False):
    """Run on 8 NeuronCores; returns (out [16,1,256,256], BassKernelResults)."""
    aff = np.asarray(inputs["affinity"], dtype=np.float32)
    att = np.asarray(inputs["attention"], dtype=np.float32)
    cs = np.asarray(inputs["current_segmentation"], dtype=np.float32)
    co = np.asarray(inputs["coarse_segmentation"], dtype=np.float32)
    band = _band_matrix()

    nc = _get_nc()
    k3 = [[i * K + j for i in rows] for (j, rows), c in
          zip(CHUNK_DEFS, CHUNK_TAPS) if len(c) == 3]
    k4 = [[i * K + j for i in rows] for (j, rows), c in
          zip(CHUNK_DEFS, CHUNK_TAPS) if len(c) == 4]
    in_maps = []
    for c in range(N_CORES):
        s = slice(c * B_CORE, (c + 1) * B_CORE)
        aff_c = aff[s].reshape(B_CORE, 49, 128, 2, W)
        a3 = np.zeros((B_CORE, N3, 128, 3, 2, HWP), np.float32)
        a3[..., PAD:PAD + W] = aff_c[:, np.array(k3).ravel()].reshape(
            B_CORE, N3, 3, 128, 2, W).transpose(0, 1, 3, 2, 4, 5)
        a4 = np.zeros((B_CORE, N4, 128, 4, 2, HWP), np.float32)
        a4[..., PAD:PAD + W] = aff_c[:, np.array(k4).ravel()].reshape(
            B_CORE, N4, 4, 128, 2, W).transpose(0, 1, 3, 2, 4, 5)
        ap = np.zeros((B_CORE, 128, 4, 2, HWP), np.float32)
        ap[..., PAD:PAD + W] = att[s].reshape(
            B_CORE, 4, 128, 2, W).transpose(0, 2, 1, 3, 4)
        in_maps.append({
            "aff3": a3,
            "aff4": a4,
            "attention": ap,
            "current_segmentation": np.ascontiguousarray(cs[s]).reshape(
                B_CORE, 128, 2, W),
            "coarse_segmentation": np.ascontiguousarray(co[s]).reshape(
                B_CORE, 128, 2, W),
            "band": band,
        })
    last_err = None
    for attempt in range(3):
        try:
            res = run_bass_kernel_spmd(nc, in_maps, list(range(N_CORES)),
                                       trace=trace)
            break
        except Exception as e:  # transient NRT_EXEC_UNIT_UNRECOVERABLE flakes
            last_err = e
            import time
            time.sleep(10)
    else:
        raise last_err
    full = np.concatenate(
        [res.results[c]["out"].reshape(B_CORE, 1, H, W) for c in range(N_CORES)],
        axis=0)
    return full, res


def kernel(**inputs) -> np.ndarray:
    out, _ = run(inputs, trace=False)
    return out


# revision 18
# speedup vs baseline: 1.0021x; 1.0021x over previous
"""DYSPN attention-conv kernel for Trainium2 (8 NeuronCores, batch-parallel).

Row-pair layout (partition p = image rows {2p,2p+1}); affinity host-repacked
column-major (chunk = upper/lower half of one 7x7-window column) and padded
to [128, cn, 2, 264]. DVE: z = att*aff -> fp8e4m3. ACT: |z| -> fp8.
PE: U/A/T fp8 band matmuls, paired taps via DoubleRow (2 contraction
rows/cycle, stacked per-tap band shifts). DVE: deferred epilogue with
approximate reciprocal. out = r*((T+att3)*cs - (U+att3)*co) + co,
r = 1/(A+att3+eps)."""
import sys

sys.path.insert(0, "/opt/trn_rl_repo")

import numpy as np

import concourse.bass as bass  # noqa: F401
import concourse.tile as tile
from concourse import bacc, mybir
from concourse.bass_utils import run_bass_kernel_spmd

FP32 = mybir.dt.float32
FP32R = mybir.dt.float32r
FP8 = mybir.dt.float8e4
DR = mybir.MatmulPerfMode.DoubleRow

N_CORES = 8
B_FULL = 16
B_CORE = B_FULL // N_CORES
H = W = 256
K = 7
PAD = 4
HWP = 2 * PAD + W
BANDW = 136
C0 = 4
EPS = 1e-6
EPI_AT = 3
EPI2_AT = 8

_INDEX = np.array([0, 0, 0, 0, 0, 0, 0,
                   0, 1, 1, 1, 1, 1, 0,
                   0, 1, 2, 2, 2, 1, 0,
                   0, 1, 2, 3, 2, 1, 0,
                   0, 1, 2, 2, 2, 1, 0,
                   0, 1, 1, 1, 1, 1, 0,
                   0, 0, 0, 0, 0, 0, 0], dtype=np.int64).reshape(7, 7)

CHUNK_DEFS = []
for j in range(K):
    CHUNK_DEFS.append((j, [0, 1, 2]))
    CHUNK_DEFS.append((j, [i for i in [3, 4, 5, 6] if not (i == 3 and j == 3)]))
CHUNK_TAPS = [[(i, int(_INDEX[i, j]), 3 - i) for i in rows]
              for j, rows in CHUNK_DEFS]
N3 = sum(1 for c in CHUNK_TAPS if len(c) == 3)
N4 = sum(1 for c in CHUNK_TAPS if len(c) == 4)


def _band_matrix() -> np.ndarray:
    band = np.zeros((128, BANDW), dtype=np.float32)
    for p in range(128):
        band[p, p + C0] = 1.0
    return band


def _build():
    nc = bacc.Bacc("TRN2", target_bir_lowering=False, debug=False,
                   num_devices=N_CORES)
    aff3 = nc.dram_tensor("aff3", [B_CORE, N3, 128, 3, 2, HWP], FP32,
                          kind="ExternalInput").ap()
    aff4 = nc.dram_tensor("aff4", [B_CORE, N4, 128, 4, 2, HWP], FP32,
                          kind="ExternalInput").ap()
    att = nc.dram_tensor("attention", [B_CORE, 128, 4, 2, HWP], FP32,
                         kind="ExternalInput").ap()
    cs = nc.dram_tensor("current_segmentation", [B_CORE, 128, 2, W], FP32,
                        kind="ExternalInput").ap()
    co = nc.dram_tensor("coarse_segmentation", [B_CORE, 128, 2, W], FP32,
                        kind="ExternalInput").ap()
    band = nc.dram_tensor("band", [128, BANDW], FP32, kind="ExternalInput").ap()
    out = nc.dram_tensor("out", [B_CORE, 128, 2, W], FP32,
                         kind="ExternalOutput").ap()

    with tile.TileContext(nc) as tc:
        with tc.tile_pool(name="const", bufs=1) as cpool, \
             tc.tile_pool(name="a3", bufs=8) as ap3, \
             tc.tile_pool(name="a4", bufs=7) as ap4, \
             tc.tile_pool(name="f3", bufs=9) as fp3, \
             tc.tile_pool(name="f4", bufs=8) as fp4, \
             tc.tile_pool(name="azp", bufs=4) as azpool, \
             tc.tile_pool(name="inp", bufs=2) as ipool, \
             tc.tile_pool(name="ep", bufs=2) as epool, \
             tc.tile_pool(name="ps", bufs=2, space="PSUM") as pspool:

            bandf = cpool.tile([128, BANDW], FP32)
            nc.scalar.dma_start(out=bandf[:], in_=band[:, :])
            bandr = cpool.tile([128, BANDW], FP32R)
            nc.vector.tensor_copy(bandr[:], bandf[:])
            identr = bandr[:, C0:C0 + 128]
            bandf8 = cpool.tile([128, BANDW], FP8)
            nc.vector.tensor_copy(bandf8[:], bandf[:])

            def ident8(s):
                return bandf8[:, C0 + s:C0 + s + 128]

            wpairs = {}
            for sp in [(0, 0), (1, 0), (2, 1), (-1, -2), (0, -1)]:
                wt = cpool.tile([128, 2, 128], FP8,
                                tag=f"w{sp[0]}_{sp[1]}")
                nc.vector.tensor_copy(wt[:, 0],
                                      bandf[:, C0 + sp[0]:C0 + sp[0] + 128])
                nc.vector.tensor_copy(wt[:, 1],
                                      bandf[:, C0 + sp[1]:C0 + sp[1] + 128])
                wpairs[sp] = wt

            st = {}

            def epilogue1(img, pool=False):
                # pool=True runs the elementwise chain on the otherwise-idle
                # Pool engine (via an ACT PSUM->SBUF copy of U, Pool has no
                # PSUM port) to take load off the bottleneck DVE stream;
                # only safe when epilogue2 is deferred far enough that the
                # slow Pool ops cannot head-of-line-block the DVE queue.
                s = st[img]
                e = epool.tile([128, 2, W], FP32, tag="e")
                s["e"] = e
                nc.scalar.activation(e[:], s["A"][:],
                                     mybir.ActivationFunctionType.Copy,
                                     bias=EPS)
                nc.vector.reciprocal_approx_fast(e[:], e[:])
                m2 = epool.tile([128, 2, W], FP32, tag="m2")
                s["m2"] = m2
                if pool:
                    u = epool.tile([128, 2, W], FP32, tag="u")
                    nc.scalar.activation(u[:], s["U"][:],
                                         mybir.ActivationFunctionType.Copy)
                    nc.gpsimd.tensor_mul(m2[:], u[:], s["cot"][:])
                    nc.gpsimd.tensor_mul(m2[:], m2[:], e[:])
                    nc.gpsimd.tensor_sub(m2[:], m2[:], s["cot"][:])
                    nc.gpsimd.tensor_mul(e[:], e[:], s["cst"][:])
                else:
                    nc.vector.tensor_mul(m2[:], s["U"][:], s["cot"][:])
                    nc.vector.tensor_mul(m2[:], m2[:], e[:])
                    nc.vector.tensor_sub(m2[:], m2[:], s["cot"][:])
                    # csr = cs/denom now (only needs e + cs) so epilogue2
                    # is two short ops per half, overlapped with the store
                    nc.vector.tensor_mul(e[:], e[:], s["cst"][:])

            def epilogue2(img):
                s = st[img]
                e, m2 = s["e"], s["m2"]
                m1 = epool.tile([128, 2, W], FP32, tag="m1")
                for h in (0, 1):
                    nc.vector.tensor_mul(m1[:, h], s["T"][:, h], e[:, h])
                    nc.vector.tensor_sub(m1[:, h], m1[:, h], m2[:, h])
                    nc.scalar.dma_start(out=out[img, :, h], in_=m1[:, h])

            attfs = []
            for img in range(B_CORE):
                attf = ipool.tile([128, 4, 2, HWP], FP32, tag="attf")
                nc.gpsimd.dma_start(out=attf[:], in_=att[img])
                cst = ipool.tile([128, 2, W], FP32, tag="cst")
                nc.gpsimd.dma_start(out=cst[:], in_=cs[img])
                cot = ipool.tile([128, 2, W], FP32, tag="cot")
                nc.gpsimd.dma_start(out=cot[:], in_=co[img])
                att3r = ipool.tile([128, 2, W], FP32R, tag="att3r")
                nc.scalar.activation(att3r[:], attf[:, 3, :, PAD:PAD + W],
                                     mybir.ActivationFunctionType.Copy)
                attfs.append(attf)
                psU = pspool.tile([128, 2, W], FP32, tag="U")
                psA = pspool.tile([128, 2, W], FP32, tag="A")
                psT = pspool.tile([128, 2, W], FP32, tag="T")
                st[img] = {"U": psU, "A": psA, "T": psT, "cst": cst,
                           "cot": cot}
                nc.tensor.matmul(out=psU[:], lhsT=identr, rhs=att3r[:],
                                 start=True, stop=False)
                nc.tensor.matmul(out=psA[:], lhsT=identr, rhs=att3r[:],
                                 start=True, stop=False)
                nc.tensor.matmul(out=psT[:], lhsT=identr, rhs=att3r[:],
                                 start=True, stop=False)

            for img in range(B_CORE):
                attf = attfs[img]
                psU = st[img]["U"]
                psA = st[img]["A"]
                psT = st[img]["T"]

                afts, zf8s = [], []
                i3 = i4 = 0
                for ci, taps in enumerate(CHUNK_TAPS):
                    if len(taps) == 3:
                        aft = ap3.tile([128, 3, 2, HWP], FP32, tag="a")
                        zf8 = fp3.tile([128, 3, 2, HWP], FP8, tag="f")
                        nc.sync.dma_start(out=aft[:], in_=aff3[img, i3])
                        i3 += 1
                    else:
                        aft = ap4.tile([128, 4, 2, HWP], FP32, tag="a")
                        zf8 = fp4.tile([128, 4, 2, HWP], FP8, tag="f")
                        nc.sync.dma_start(out=aft[:], in_=aff4[img, i4])
                        i4 += 1
                    afts.append(aft)
                    zf8s.append(zf8)

                for ci, taps in enumerate(CHUNK_TAPS):
                    j, _ = CHUNK_DEFS[ci]
                    dx = 3 - j
                    cn = len(taps)
                    aft, zf8 = afts[ci], zf8s[ci]
                    final = ci == len(CHUNK_TAPS) - 1
                    a = 0
                    while a < cn:
                        b = a + 1
                        while b < cn and taps[b][1] == taps[a][1]:
                            b += 1
                        r = taps[a][1]
                        nc.vector.tensor_tensor(
                            out=zf8[:, a:b],
                            in0=aft[:, a:b],
                            in1=attf[:, r].unsqueeze(1).broadcast_to(
                                [128, b - a, 2, HWP]),
                            op=mybir.AluOpType.mult)
                        a = b
                    azt = azpool.tile([128, 4, 2, W], FP8, tag="az")
                    nc.scalar.activation(azt[:, 0:cn],
                                         zf8[:, :, :, PAD:PAD + W],
                                         mybir.ActivationFunctionType.Abs)

                    def sum_pairs(ps, src, off, stop, cn=cn):
                        mms = []
                        for a0 in range(0, cn - 1, 2):
                            for h in (0, 1):
                                mms.append((ps[:, h], wpairs[(0, 0)][:],
                                            src[:, a0:a0 + 2, h,
                                                off:off + W], DR))
                        if cn % 2:
                            mms.append((ps[:, :, :], ident8(0),
                                        src[:, cn - 1, :, off:off + W],
                                        None))
                        for n, (o, w, rhs, pm) in enumerate(mms):
                            nc.tensor.matmul(out=o, lhsT=w, rhs=rhs,
                                             perf_mode=pm, start=False,
                                             stop=(stop and n == len(mms) - 1),
                                             skip_group_check=True)

                    def mm_t(stop, taps=taps, dx=dx, zf8=zf8):
                        mms = []
                        odd = [n for n, (i, r, dy) in enumerate(taps)
                               if dy % 2]
                        ev = [n for n, (i, r, dy) in enumerate(taps)
                              if dy % 2 == 0]
                        xo = PAD + dx
                        if dx % 2 == 0:
                            for h in (0, 1):
                                sab = tuple((taps[n][2] - 1) // 2 if h == 0
                                            else (taps[n][2] + 1) // 2
                                            for n in odd)
                                mms.append((psT[:, h], wpairs[sab][:],
                                            zf8[:, odd[0]:odd[1] + 1:2,
                                                1 - h, xo:xo + W], DR))
                            if len(ev) == 2:
                                sab = tuple(taps[n][2] // 2 for n in ev)
                                for h in (0, 1):
                                    mms.append((psT[:, h], wpairs[sab][:],
                                                zf8[:, ev[0]:ev[1] + 1:2,
                                                    h, xo:xo + W], DR))
                            else:
                                s = taps[ev[0]][2] // 2
                                mms.append((psT[:, :, :], ident8(s),
                                            zf8[:, ev[0], :, xo:xo + W],
                                            None))
                        else:
                            for n, (i, r, dy) in enumerate(taps):
                                if dy % 2 == 0:
                                    mms.append((psT[:, :, :],
                                                ident8(dy // 2),
                                                zf8[:, n, :, xo:xo + W],
                                                None))
                                else:
                                    for h in (0, 1):
                                        s = ((dy - 1) // 2 if h == 0
                                             else (dy + 1) // 2)
                                        mms.append((psT[:, h], ident8(s),
                                                    zf8[:, n, 1 - h,
                                                        xo:xo + W], None))
                        for n, (o, w, rhs, pm) in enumerate(mms):
                            nc.tensor.matmul(out=o, lhsT=w, rhs=rhs,
                                             perf_mode=pm, start=False,
                                             stop=(stop and n == len(mms) - 1),
                                             skip_group_check=True)

                    if final and img == B_CORE - 1:
                        sum_pairs(psU, zf8, PAD, True)
                        sum_pairs(psA, azt, 0, True)
                        epilogue1(img)
                        mm_t(True)
                        epilogue2(img)
                    else:
                        sum_pairs(psU, zf8, PAD, final)
                        mm_t(final)
                        sum_pairs(psA, azt, 0, final)
                    if ci == EPI_AT and img > 0:
                        epilogue1(img - 1, pool=True)
                    if ci == EPI2_AT and img > 0:
                        epilogue2(img - 1)

    nc.compile()
    return nc


_NC_CACHE = None


def _get_nc():
    global _NC_CACHE
    if _NC_CACHE is None:
        _NC_CACHE = _build()
    return _NC_CACHE


def run(inputs: dict, trace: bool = False):
    """Run on 8 NeuronCores; returns (out [16,1,256,256], BassKernelResults)."""
    aff = np.asarray(inputs["affinity"], dtype=np.float32)
    att = np.asarray(inputs["attention"], dtype=np.float32)
    cs = np.asarray(inputs["current_segmentation"], dtype=np.float32)
    co = np.asarray(inputs["coarse_segmentation"], dtype=np.float32)
    band = _band_matrix()

    nc = _get_nc()
    k3 = [[i * K + j for i in rows] for (j, rows), c in
          zip(CHUNK_DEFS, CHUNK_TAPS) if len(c) == 3]
    k4 = [[i * K + j for i in rows] for (j, rows), c in
          zip(CHUNK_DEFS, CHUNK_TAPS) if len(c) == 4]
    in_maps = []
    for c in range(N_CORES):
        s = slice(c * B_CORE, (c + 1) * B_CORE)
        aff_c = aff[s].reshape(B_CORE, 49, 128, 2, W)
        a3 = np.zeros((B_CORE, N3, 128, 3, 2, HWP), np.float32)
        a3[..., PAD:PAD + W] = aff_c[:, np.array(k3).ravel()].reshape(
            B_CORE, N3, 3, 128, 2, W).transpose(0, 1, 3, 2, 4, 5)
        a4 = np.zeros((B_CORE, N4, 128, 4, 2, HWP), np.float32)
        a4[..., PAD:PAD + W] = aff_c[:, np.array(k4).ravel()].reshape(
            B_CORE, N4, 4, 128, 2, W).transpose(0, 1, 3, 2, 4, 5)
        ap = np.zeros((B_CORE, 128, 4, 2, HWP), np.float32)
        ap[..., PAD:PAD + W] = att[s].reshape(
            B_CORE, 4, 128, 2, W).transpose(0, 2, 1, 3, 4)
        in_maps.append({
            "aff3": a3,
            "aff4": a4,
            "attention": ap,
            "current_segmentation": np.ascontiguousarray(cs[s]).reshape(
                B_CORE, 128, 2, W),
            "coarse_segmentation": np.ascontiguousarray(co[s]).reshape(
                B_CORE, 128, 2, W),
            "band": band,
        })
    last_err = None
    for attempt in range(3):
        try:
            res = run_bass_kernel_spmd(nc, in_maps, list(range(N_CORES)),
                                       trace=trace)
            break
        except Exception as e:  # transient NRT_EXEC_UNIT_UNRECOVERABLE flakes
            last_err = e
            import time
            time.sleep(10)
    else:
        raise last_err
    full = np.concatenate(
        [res.results[c]["out"].reshape(B_CORE, 1, H, W) for c in range(N_CORES)],
        axis=0)
    return full, res


def kernel(**inputs) -> np.ndarray:
    out, _ = run(inputs, trace=False)
    return out
